# revision 12
# baseline (speedup 1.0000x reference)
"""Causal self-attention (RoPE, 16 heads, dim 2048, B=2, S=2048) on 8 trn2 cores.

Sharding: pure head-parallel attention (2 heads/core, both batches). A single
8-rank AllToAll per iteration reshards head-columns -> sequence-rows
((batch, s-quarter) slots); each core runs the output projection for one
512-wide sequence shard.

v2 (this file) vs the fp32r baseline:
  - bf16 operands on every matmul (PE rate is 1 cycle/row either way, but
    DMA bytes, SBUF footprint, A2A payload and DVE elementwise cost halve;
    fp32 accumulation in PSUM throughout keeps rel-err ~1e-2 > margin).
  - softmax row-sums no longer burn a 512-cycle ones-matmul per k-block:
    exp blocks are accumulated elementwise on DVE (bf16 2x mode) and a
    single ones-matmul per (b, qt, head) contracts the folded partials.
  - causal q-slicing on the diagonal-band blocks (scores/exp/mask/PV/accum
    all run on the [o:512] live columns only).
  - weights/cos/sin/band/ones are loaded once (persistent across the
    n_iters chain) on the ACT hwdge queue, x streams per-iter on sync.
  - cross-iteration software pipeline: the AllToAll of iter i runs during
    the qkv phase of iter i+1, and the output projection of iter i is
    emitted as PE *filler* interleaved into the attention rounds of iter
    i+1 — the PE never waits on the collective and the ACT exp overhang
    (612ns/block vs 427ns of PE per block) hides under filler matmuls.
  - rope: sign folded into the sin table (sinN = [-sin_lo; sin_hi]) so a
    head's rotation is 4 DVE ops (1 full mul, 2 shifted half muls, 1 add).

PSUM map (tags t0..t7, one 2KB bank each):
  qkv phase:  t0,t1 = pq(h) | t2,t3 = pk(h) | t4..t7 = pv(ss) [128,256]
  attn phase: t0,t1 = pscore(h) | t2,t3 = out-proj pout parity
              t4,t5 = po(h) | t6,t7 = pS(h) replicated row sums
"""

import numpy as np

import concourse.bacc as bacc
import concourse.mybir as mybir
import concourse.tile as tile
from concourse.bass_utils import run_bass_kernel_spmd

DIM = 2048
H = 16
D = 128
B, S = 2, 2048
N_CORES = 8
HPC = H // N_CORES  # 2 heads per core
QT = 512  # q tile (free dim)
NQT = S // QT  # 4
NMC = DIM // 128  # 16 contraction chunks
SCALE = float(D) ** -0.5

F32 = mybir.dt.float32
BF16 = mybir.dt.bfloat16
NPBF16 = mybir.dt.np(BF16)


def _rope_tables():
    inv_freq = 1.0 / (10000.0 ** (np.arange(0, D, 2, dtype=np.float32) / D))
    t = np.arange(S, dtype=np.float32)
    freqs = t[:, None] * inv_freq[None, :]
    emb = np.concatenate([freqs, freqs], axis=-1)  # [S, D]
    cosT = np.ascontiguousarray(np.cos(emb).T.astype(np.float32))
    sinT = np.sin(emb).T.astype(np.float32)
    sinNT = sinT.copy()
    sinNT[: D // 2] *= -1.0  # fold the rotate-half sign into the table
    return cosT, np.ascontiguousarray(sinNT)


def _band_mask():
    # band[dk, j] = 1.0 if dk <= j - 384 else 0; diagonal block with offset
    # o = k0 - q0 uses band[:, 384-o : 384-o+W]
    dk = np.arange(128)[:, None]
    j = np.arange(384 + QT)[None, :]
    return (dk <= j - 384).astype(np.float32)


def build(n_iters: int = 1, single_core: bool = False):
    nc = bacc.Bacc(
        "TRN2",
        target_bir_lowering=False,
        debug=False,
        num_devices=1 if single_core else N_CORES,
    )

    xP = nc.dram_tensor("xP", [B, NMC, NQT, 128, QT], BF16, kind="ExternalInput").ap()
    wq = nc.dram_tensor("wq", [128, NMC, HPC * D], BF16, kind="ExternalInput").ap()
    wk = nc.dram_tensor("wk", [128, NMC, HPC * D], BF16, kind="ExternalInput").ap()
    wv = nc.dram_tensor("wv", [128, NMC, HPC * D], BF16, kind="ExternalInput").ap()
    woP = nc.dram_tensor("woP", [NMC, 128, NMC, 128], BF16, kind="ExternalInput").ap()
    cosT = nc.dram_tensor("cosT", [D, S], BF16, kind="ExternalInput").ap()
    sinNT = nc.dram_tensor("sinNT", [D, S], BF16, kind="ExternalInput").ap()
    band = nc.dram_tensor("band", [128, 384 + QT], BF16, kind="ExternalInput").ap()
    outT = nc.dram_tensor("outT", [DIM, QT], F32, kind="ExternalOutput").ap()

    with tile.TileContext(nc) as tc:
        _body(tc, n_iters, xP, wq, wk, wv, woP, cosT, sinNT, band, outT,
              single_core=single_core)
    nc.compile()
    return nc


def _a2a(nc, a2a_in, a2a_out, single_core):
    if single_core:
        nc.sync.dma_start(out=a2a_out, in_=a2a_in)
    else:
        nc.gpsimd.collective_compute(
            "AllToAll",
            mybir.AluOpType.bypass,
            replica_groups=[list(range(N_CORES))],
            ins=[a2a_in.opt()],
            outs=[a2a_out.opt()],
        )


def _body(tc, n_iters, xP, wq, wk, wv, woP, cosT, sinNT, band, outT, single_core=False):
    nc = tc.nc
    from contextlib import ExitStack

    with ExitStack() as ctx:
        const = ctx.enter_context(tc.tile_pool(name="const", bufs=1))
        dram = ctx.enter_context(tc.tile_pool(name="dram", bufs=1, space="DRAM"))

        # ---- persistent constants/weights, loaded once on the ACT queue ----
        band_t = const.tile([128, 384 + QT], BF16, tag="band")
        nc.scalar.dma_start(out=band_t, in_=band)
        ones_f32 = const.tile([128, 128], F32, tag="ones_f32")
        nc.vector.memset(ones_f32, 1.0)
        ones_t = const.tile([128, 128], BF16, tag="ones")
        nc.vector.tensor_copy(out=ones_t, in_=ones_f32)
        cos_t = const.tile([D, S], BF16, tag="cos")
        sin_t = const.tile([D, S], BF16, tag="sinN")
        nc.scalar.dma_start(out=cos_t, in_=cosT)
        nc.scalar.dma_start(out=sin_t, in_=sinNT)
        wq_t, wk_t, wv_t = [], [], []
        for nm, src, lst in (("wv", wv, wv_t), ("wq", wq, wq_t), ("wk", wk, wk_t)):
            for mc in range(NMC):
                t = const.tile([128, HPC * D], BF16, tag=f"{nm}{mc}", name=f"{nm}{mc}")
                nc.scalar.dma_start(out=t, in_=src[:, mc, :])
                lst.append(t)
        wo_t = []
        for oc in range(NMC):
            t = const.tile([128, NMC, 128], BF16, tag=f"wo{oc}", name=f"wo{oc}")
            nc.scalar.dma_start(out=t, in_=woP[oc])
            wo_t.append(t)

        a2a_ins, a2a_outs = [], []
        for p in range(2):
            a2a_ins.append(dram.tile([N_CORES, HPC * D, QT], BF16,
                                     tag=f"a2a_in{p}", name=f"a2a_in{p}"))
            a2a_outs.append(dram.tile([N_CORES, HPC * D, QT], BF16,
                                      tag=f"a2a_out{p}", name=f"a2a_out{p}"))

        xp = ctx.enter_context(tc.tile_pool(name="xp", bufs=18))
        qkv = ctx.enter_context(tc.tile_pool(name="qkv", bufs=1))
        expp = ctx.enter_context(tc.tile_pool(name="expp", bufs=3))
        attn = ctx.enter_context(tc.tile_pool(name="attn", bufs=2))
        accp = ctx.enter_context(tc.tile_pool(name="accp", bufs=2))
        recvp = ctx.enter_context(tc.tile_pool(name="recvp", bufs=1))
        ps = ctx.enter_context(tc.tile_pool(name="ps", bufs=1, space="PSUM"))

        recv_t = recvp.tile([128, NMC, QT], BF16, tag="recv", name="recv")

        # ---- out-projection of iteration `it`, as a list of thunks ----
        # each thunk emits one PE matmul; every 16th completes an oc group
        # and also emits the drain copy (DVE) + outT DMA. pout alternates
        # PSUM tags t2/t3 so consecutive oc groups never WAR-stall.
        def make_outproj_thunks():
            thunks = []
            state = {}

            def mk(oc, cc):
                def thunk():
                    if cc == 0:
                        state["pout"] = ps.tile(
                            [128, QT], F32, tag=f"t{2 + oc % 2}", name="pout"
                        )
                    nc.tensor.matmul(
                        state["pout"], wo_t[oc][:, cc, :], recv_t[:, cc, :],
                        start=(cc == 0), stop=(cc == NMC - 1),
                    )
                    if cc == NMC - 1:
                        res = attn.tile([128, QT], F32, tag="res")
                        nc.scalar.copy(out=res, in_=state["pout"])
                        nc.sync.dma_start(
                            out=outT[oc * 128 : (oc + 1) * 128, :], in_=res
                        )
                return thunk

            for oc in range(NMC):
                for cc in range(NMC):
                    thunks.append(mk(oc, cc))
            return thunks

        def emit_filler(filler, k):
            for _ in range(k):
                if not filler:
                    return
                filler.pop(0)()

        # ---- qkv projection + rope for batch b ----
        def qkv_phase(b, qr_ts, kr_ts, v_ts):
            for st in range(NQT):
                pq = [ps.tile([128, QT], F32, tag=f"t{h}", name=f"pq{h}")
                      for h in range(HPC)]
                pk = [ps.tile([128, QT], F32, tag=f"t{2 + h}", name=f"pk{h}")
                      for h in range(HPC)]
                pv = [ps.tile([128, HPC * D], F32, tag=f"t{4 + i}", name=f"pv{i}")
                      for i in range(4)]
                # v first: these matmuls run while the previous s-tile's rope
                # drains the q/k psum banks on DVE; x stays resident for q/k
                xts = []
                for mc in range(NMC):
                    xt = xp.tile([128, QT], BF16, tag="x", name="xt")
                    nc.sync.dma_start(out=xt, in_=xP[b, mc, st])
                    xts.append(xt)
                    for ss in range(4):
                        nc.tensor.matmul(
                            pv[ss],
                            xt[:, ss * 128 : (ss + 1) * 128], wv_t[mc],
                            start=(mc == 0), stop=(mc == NMC - 1),
                        )
                for ss in range(4):
                    nc.scalar.copy(out=v_ts[st][:, ss, :], in_=pv[ss])
                for mc in range(NMC):
                    for h in range(HPC):
                        nc.tensor.matmul(
                            pq[h], wq_t[mc][:, h * D : (h + 1) * D], xts[mc],
                            start=(mc == 0), stop=(mc == NMC - 1),
                        )
                        nc.tensor.matmul(
                            pk[h], wk_t[mc][:, h * D : (h + 1) * D], xts[mc],
                            start=(mc == 0), stop=(mc == NMC - 1),
                        )
                # rope: dst = src*cos + shifted(src)*sinN  (sign is in sinN).
                # PSUM reads must stay on DVE (Pool has no PSUM port); the
                # SBUF-only final add goes to the otherwise-idle Pool engine.
                cs = slice(st * QT, st * QT + QT)
                for h in range(HPC):
                    for src, dst in ((pq[h], qr_ts[st]), (pk[h], kr_ts[st])):
                        tmp = attn.tile([128, 2, QT], F32, tag="ropetmp")
                        nc.vector.tensor_mul(tmp[:, 0], src, cos_t[:, cs])
                        nc.vector.tensor_mul(tmp[0:64, 1], src[64:128], sin_t[0:64, cs])
                        nc.vector.tensor_mul(tmp[64:128, 1], src[0:64], sin_t[64:128, cs])
                        nc.gpsimd.tensor_add(dst[:, h, :], tmp[:, 0], tmp[:, 1])

        # ---- attention for batch b; heads interleaved, PV trails by one
        # k-block; `filler` thunks keep the PE fed while ACT exps ----
        def attn_phase(b, par, qr_ts, kr_ts, v_ts, filler):
            for qt in range(NQT):
                n_kb = 4 * qt + 4
                po = [ps.tile([128, QT], F32, tag=f"t{4 + h}", name=f"po{h}")
                      for h in range(HPC)]
                acc = [accp.tile([128, QT], BF16, tag=f"acc{h}", name=f"acc{h}")
                       for h in range(HPC)]

                def flush(h, e, kb, o):
                    nc.tensor.matmul(
                        po[h][:, o:],
                        v_ts[kb // 4][:, kb % 4, h * D : (h + 1) * D], e[:, o:],
                        start=(kb == 0), stop=(kb == n_kb - 1),
                    )

                prev = {}
                for kb in range(n_kb):
                    diag = kb - (n_kb - 4)
                    o = max(diag, 0) * 128
                    cur = {}
                    for h in range(HPC):
                        pscore = ps.tile([128, QT], F32, tag=f"t{h}",
                                         name=f"pscore{h}")
                        nc.tensor.matmul(
                            pscore[:, o:],
                            kr_ts[kb // 4][:, h, (kb % 4) * 128 : (kb % 4 + 1) * 128],
                            qr_ts[qt][:, h, o:],
                            start=True, stop=True,
                        )
                        e = expp.tile([128, QT], BF16, tag=f"e{h}", name=f"e{h}",
                                      bufs=3)
                        nc.scalar.activation(
                            out=e[:, o:], in_=pscore[:, o:],
                            func=mybir.ActivationFunctionType.Exp, scale=SCALE,
                        )
                        if diag >= 0:
                            # only columns [o, o+128) of the block straddle
                            # the diagonal; q >= o+128 is fully allowed
                            nc.gpsimd.tensor_mul(
                                e[:, o : o + 128], e[:, o : o + 128],
                                band_t[:, 384:512],
                            )
                        if kb == 0:
                            if qt == 0:
                                nc.vector.tensor_copy(out=acc[h], in_=e)
                        elif kb == 1 and qt > 0:
                            # fused init: both blocks are full-width off-diag
                            nc.vector.tensor_add(acc[h], prev[h][0], e)
                        else:
                            nc.vector.tensor_add(
                                acc[h][:, o:], acc[h][:, o:], e[:, o:]
                            )
                        cur[h] = (e, kb, o)
                    for h in range(HPC):
                        if prev:
                            flush(h, *prev[h])
                    emit_filler(filler, 3)
                    prev = cur
                for h in range(HPC):
                    flush(h, *prev[h])

                # row sums via ones-matmul with a FULL [128,128] ones lhsT:
                # same 512-cycle cost, but the sum arrives replicated across
                # all 128 partitions — the reciprocal is then already
                # broadcast and no pb outer-product matmul is needed.
                for h in range(HPC):
                    pS = ps.tile([128, QT], F32, tag=f"t{6 + h}", name=f"pS{h}")
                    nc.tensor.matmul(pS, ones_t, acc[h], start=True, stop=True)
                    recip = attn.tile([128, QT], BF16, tag="recip")
                    with nc.allow_low_precision(reason="bf16 recip, feeds bf16 mul"):
                        nc.vector.reciprocal(out=recip, in_=pS)
                    a32 = attn.tile([128, QT], BF16, tag="a32")
                    nc.scalar.copy(out=a32, in_=po[h])
                    aout = attn.tile([128, QT], BF16, tag="aout")
                    nc.gpsimd.tensor_mul(aout, a32, recip)
                    nc.sync.dma_start(
                        out=a2a_ins[par][b * NQT + qt, h * D : (h + 1) * D, :],
                        in_=aout,
                    )
                emit_filler(filler, 2)

        # ---- iteration chain: A2A(it) overlaps qkv(it+1); outproj(it-1)
        # is the attention filler of iteration it ----
        qr_ts = [qkv.tile([128, HPC, QT], BF16, tag=f"qr{st}", name=f"qr{st}")
                 for st in range(NQT)]
        kr_ts = [qkv.tile([128, HPC, QT], BF16, tag=f"kr{st}", name=f"kr{st}")
                 for st in range(NQT)]
        v_ts = [qkv.tile([128, 4, HPC * D], BF16, tag=f"v{st}", name=f"v{st}")
                for st in range(NQT)]

        for it in range(n_iters):
            par = it % 2
            filler = make_outproj_thunks() if it > 0 else []
            for b in range(B):
                qkv_phase(b, qr_ts, kr_ts, v_ts)
                attn_phase(b, par, qr_ts, kr_ts, v_ts, filler)
            emit_filler(filler, len(filler))
            _a2a(nc, a2a_ins[par], a2a_outs[par], single_core)
            for cc in range(NMC):
                nc.sync.dma_start(
                    out=recv_t[:, cc, :],
                    in_=a2a_outs[par][cc // 2, (cc % 2) * 128 : (cc % 2) * 128 + 128, :],
                )
        # epilogue: out-projection of the final iteration
        emit_filler(make_outproj_thunks(), NMC * NMC)


_CACHE = {}


def _get_built(n_iters=1):
    if n_iters not in _CACHE:
        _CACHE[n_iters] = build(n_iters)
    return _CACHE[n_iters]


def _fallback_numpy(x, w_qkv, w_out, mask):
    B_, S_, _ = x.shape
    qkv = x @ w_qkv
    qkv = qkv.reshape(B_, S_, 3, H, D).transpose(2, 0, 3, 1, 4)
    q, k, v = qkv[0], qkv[1], qkv[2]
    cosT, sinNT = _rope_tables()
    sinT = sinNT.copy()
    sinT[: D // 2] *= -1.0
    cos, sin = cosT.T[None, None], sinT.T[None, None]

    def rot(t):
        return np.concatenate([-t[..., D // 2 :], t[..., : D // 2]], axis=-1)

    q = q * cos + rot(q) * sin
    k = k * cos + rot(k) * sin
    score = np.einsum("bhqd,bhkd->bhqk", q, k) * SCALE
    score = np.where(mask == 0, -np.inf, score)
    score = score - score.max(axis=-1, keepdims=True)
    e = np.exp(score)
    attn = e / e.sum(axis=-1, keepdims=True)
    out = np.einsum("bhqk,bhkd->bhqd", attn, v)
    out = out.transpose(0, 2, 1, 3).reshape(B_, S_, H * D)
    return (out @ w_out).astype(np.float32)


def make_in_maps(x, w_qkv, w_out):
    cosT, sinNT = _rope_tables()
    band = _band_mask()
    # x pre-tiled: [B, mc, st, 128, 512], contiguous per tile
    xT = x.transpose(0, 2, 1)  # [B, DIM, S]
    xP = np.ascontiguousarray(
        xT.reshape(B, NMC, 128, NQT, QT).transpose(0, 1, 3, 2, 4)
    ).astype(NPBF16)
    # w_out pre-swizzled: [oc, p, cc, o] so each [128, 16*128] load is contiguous
    woP = np.ascontiguousarray(
        w_out.reshape(NMC, 128, NMC, 128).transpose(2, 1, 0, 3)
    ).astype(NPBF16)
    cosT = cosT.astype(NPBF16)
    sinNT = sinNT.astype(NPBF16)
    band = band.astype(NPBF16)
    in_maps = []
    for c in range(N_CORES):
        heads = [HPC * c + i for i in range(HPC)]

        def wslice(base):
            w = np.concatenate(
                [w_qkv[:, base + h * D : base + (h + 1) * D] for h in heads], axis=1
            )  # [DIM, 256]
            # -> [p, mc, 256] contiguous per partition
            return np.ascontiguousarray(
                w.reshape(NMC, 128, HPC * D).transpose(1, 0, 2)
            ).astype(NPBF16)

        in_maps.append(
            {
                "xP": xP,
                "wq": wslice(0),
                "wk": wslice(DIM),
                "wv": wslice(2 * DIM),
                "woP": woP,
                "cosT": cosT,
                "sinNT": sinNT,
                "band": band,
            }
        )
    return in_maps


def assemble_output(results):
    out = np.zeros((B, S, DIM), np.float32)
    for j in range(N_CORES):
        b, sq = j // NQT, j % NQT
        out[b, sq * QT : (sq + 1) * QT, :] = results[j]["outT"].T
    return out


def kernel(x, w_qkv, w_out, mask):
    x = np.asarray(x, dtype=np.float32)
    w_qkv = np.asarray(w_qkv, dtype=np.float32)
    w_out = np.asarray(w_out, dtype=np.float32)
    mask = np.asarray(mask)
    if not np.array_equal(mask != 0, np.tril(np.ones((S, S), bool))):
        return _fallback_numpy(x, w_qkv, w_out, mask)
    nc = _get_built(1)
    res = run_bass_kernel_spmd(nc, make_in_maps(x, w_qkv, w_out), list(range(N_CORES)))
    return assemble_output(res.results)


# revision 14
# speedup vs baseline: 1.0478x; 1.0478x over previous
"""Causal self-attention (RoPE, 16 heads, dim 2048, B=2, S=2048) on 8 trn2 cores.

Sharding: pure head-parallel attention (2 heads/core, both batches). A single
8-rank AllToAll per iteration reshards head-columns -> sequence-rows
((batch, s-quarter) slots); each core runs the output projection for one
512-wide sequence shard.

v2 (this file) vs the fp32r baseline:
  - bf16 operands on every matmul (PE rate is 1 cycle/row either way, but
    DMA bytes, SBUF footprint, A2A payload and DVE elementwise cost halve;
    fp32 accumulation in PSUM throughout keeps rel-err ~1e-2 > margin).
  - softmax row-sums no longer burn a 512-cycle ones-matmul per k-block:
    exp blocks are accumulated elementwise on DVE (bf16 2x mode) and a
    single ones-matmul per (b, qt, head) contracts the folded partials.
  - causal q-slicing on the diagonal-band blocks (scores/exp/mask/PV/accum
    all run on the [o:512] live columns only).
  - weights/cos/sin/band/ones are loaded once (persistent across the
    n_iters chain) on the ACT hwdge queue, x streams per-iter on sync.
  - cross-iteration software pipeline: the AllToAll of iter i runs during
    the qkv phase of iter i+1, and the output projection of iter i is
    emitted as PE *filler* interleaved into the attention rounds of iter
    i+1 — the PE never waits on the collective and the ACT exp overhang
    (612ns/block vs 427ns of PE per block) hides under filler matmuls.
  - rope: sign folded into the sin table (sinN = [-sin_lo; sin_hi]) so a
    head's rotation is 4 DVE ops (1 full mul, 2 shifted half muls, 1 add).

PSUM map (tags t0..t7, one 2KB bank each):
  qkv phase:  t0,t1 = pq(h) | t2,t3 = pk(h) | t4..t7 = pv(ss) [128,256]
  attn phase: t0,t1 = pscore(h) | t2,t3 = out-proj pout parity
              t4,t5 = po(h) | t6,t7 = pS(h) replicated row sums
"""

import numpy as np

import concourse.bacc as bacc
import concourse.mybir as mybir
import concourse.tile as tile
from concourse.bass_utils import run_bass_kernel_spmd

DIM = 2048
H = 16
D = 128
B, S = 2, 2048
N_CORES = 8
HPC = H // N_CORES  # 2 heads per core
QT = 512  # q tile (free dim)
NQT = S // QT  # 4
NMC = DIM // 128  # 16 contraction chunks
SCALE = float(D) ** -0.5

F32 = mybir.dt.float32
BF16 = mybir.dt.bfloat16
NPBF16 = mybir.dt.np(BF16)


def _rope_tables():
    inv_freq = 1.0 / (10000.0 ** (np.arange(0, D, 2, dtype=np.float32) / D))
    t = np.arange(S, dtype=np.float32)
    freqs = t[:, None] * inv_freq[None, :]
    emb = np.concatenate([freqs, freqs], axis=-1)  # [S, D]
    cosT = np.ascontiguousarray(np.cos(emb).T.astype(np.float32))
    sinT = np.sin(emb).T.astype(np.float32)
    sinNT = sinT.copy()
    sinNT[: D // 2] *= -1.0  # fold the rotate-half sign into the table
    return cosT, np.ascontiguousarray(sinNT)


def _band_mask():
    # band[dk, j] = 1.0 if dk <= j - 384 else 0; diagonal block with offset
    # o = k0 - q0 uses band[:, 384-o : 384-o+W]
    dk = np.arange(128)[:, None]
    j = np.arange(384 + QT)[None, :]
    return (dk <= j - 384).astype(np.float32)


def build(n_iters: int = 1, single_core: bool = False):
    nc = bacc.Bacc(
        "TRN2",
        target_bir_lowering=False,
        debug=False,
        num_devices=1 if single_core else N_CORES,
    )

    xP = nc.dram_tensor("xP", [B, NMC, NQT, 128, QT], BF16, kind="ExternalInput").ap()
    wq = nc.dram_tensor("wq", [128, NMC, HPC * D], BF16, kind="ExternalInput").ap()
    wk = nc.dram_tensor("wk", [128, NMC, HPC * D], BF16, kind="ExternalInput").ap()
    wv = nc.dram_tensor("wv", [128, NMC, HPC * D], BF16, kind="ExternalInput").ap()
    woP = nc.dram_tensor("woP", [NMC, 128, NMC, 128], BF16, kind="ExternalInput").ap()
    cosT = nc.dram_tensor("cosT", [D, S], BF16, kind="ExternalInput").ap()
    sinNT = nc.dram_tensor("sinNT", [D, S], BF16, kind="ExternalInput").ap()
    band = nc.dram_tensor("band", [128, 384 + QT], BF16, kind="ExternalInput").ap()
    outT = nc.dram_tensor("outT", [DIM, QT], F32, kind="ExternalOutput").ap()

    with tile.TileContext(nc) as tc:
        _body(tc, n_iters, xP, wq, wk, wv, woP, cosT, sinNT, band, outT,
              single_core=single_core)
    nc.compile()
    return nc


def _a2a(nc, a2a_in, a2a_out, single_core):
    if single_core:
        nc.sync.dma_start(out=a2a_out, in_=a2a_in)
    else:
        nc.gpsimd.collective_compute(
            "AllToAll",
            mybir.AluOpType.bypass,
            replica_groups=[list(range(N_CORES))],
            ins=[a2a_in.opt()],
            outs=[a2a_out.opt()],
        )


def _body(tc, n_iters, xP, wq, wk, wv, woP, cosT, sinNT, band, outT, single_core=False):
    nc = tc.nc
    from contextlib import ExitStack

    with ExitStack() as ctx:
        const = ctx.enter_context(tc.tile_pool(name="const", bufs=1))
        dram = ctx.enter_context(tc.tile_pool(name="dram", bufs=1, space="DRAM"))

        # ---- persistent constants/weights, loaded once on the ACT queue ----
        band_t = const.tile([128, 384 + QT], BF16, tag="band")
        nc.scalar.dma_start(out=band_t, in_=band)
        ones_f32 = const.tile([128, 128], F32, tag="ones_f32")
        nc.vector.memset(ones_f32, 1.0)
        ones_t = const.tile([128, 128], BF16, tag="ones")
        nc.vector.tensor_copy(out=ones_t, in_=ones_f32)
        cos_t = const.tile([D, S], BF16, tag="cos")
        sin_t = const.tile([D, S], BF16, tag="sinN")
        nc.scalar.dma_start(out=cos_t, in_=cosT)
        nc.scalar.dma_start(out=sin_t, in_=sinNT)
        wq_t, wk_t, wv_t = [], [], []
        for nm, src, lst in (("wv", wv, wv_t), ("wq", wq, wq_t), ("wk", wk, wk_t)):
            for mc in range(NMC):
                t = const.tile([128, HPC * D], BF16, tag=f"{nm}{mc}", name=f"{nm}{mc}")
                nc.scalar.dma_start(out=t, in_=src[:, mc, :])
                lst.append(t)
        wo_t = []
        for oc in range(NMC):
            t = const.tile([128, NMC, 128], BF16, tag=f"wo{oc}", name=f"wo{oc}")
            nc.scalar.dma_start(out=t, in_=woP[oc])
            wo_t.append(t)

        a2a_ins, a2a_outs = [], []
        for p in range(2):
            a2a_ins.append(dram.tile([N_CORES, HPC * D, QT], BF16,
                                     tag=f"a2a_in{p}", name=f"a2a_in{p}"))
            a2a_outs.append(dram.tile([N_CORES, HPC * D, QT], BF16,
                                      tag=f"a2a_out{p}", name=f"a2a_out{p}"))

        xp = ctx.enter_context(tc.tile_pool(name="xp", bufs=18))
        qkv = ctx.enter_context(tc.tile_pool(name="qkv", bufs=1))
        expp = ctx.enter_context(tc.tile_pool(name="expp", bufs=3))
        attn = ctx.enter_context(tc.tile_pool(name="attn", bufs=2))
        accp = ctx.enter_context(tc.tile_pool(name="accp", bufs=2))
        recvp = ctx.enter_context(tc.tile_pool(name="recvp", bufs=1))
        ps = ctx.enter_context(tc.tile_pool(name="ps", bufs=1, space="PSUM"))

        recv_t = recvp.tile([128, NMC, QT], BF16, tag="recv", name="recv")

        # ---- out-projection of iteration `it`, as a list of thunks ----
        # each thunk emits one PE matmul; every 16th completes an oc group
        # and also emits the drain copy (DVE) + outT DMA. pout alternates
        # PSUM tags t2/t3 so consecutive oc groups never WAR-stall.
        def make_outproj_thunks():
            thunks = []
            state = {}

            def mk(oc, cc):
                def thunk():
                    if cc == 0:
                        state["pout"] = ps.tile(
                            [128, QT], F32, tag=f"t{2 + oc % 2}", name="pout"
                        )
                    nc.tensor.matmul(
                        state["pout"], wo_t[oc][:, cc, :], recv_t[:, cc, :],
                        start=(cc == 0), stop=(cc == NMC - 1),
                    )
                    if cc == NMC - 1:
                        res = attn.tile([128, QT], F32, tag="res")
                        nc.scalar.copy(out=res, in_=state["pout"])
                        nc.sync.dma_start(
                            out=outT[oc * 128 : (oc + 1) * 128, :], in_=res
                        )
                return thunk

            for oc in range(NMC):
                for cc in range(NMC):
                    thunks.append(mk(oc, cc))
            return thunks

        def emit_filler(filler, k):
            for _ in range(k):
                if not filler:
                    return
                filler.pop(0)()

        # ---- qkv projection + rope for batch b ----
        def qkv_phase(b, qr_ts, kr_ts, v_ts):
            for st in range(NQT):
                pq = [ps.tile([128, QT], F32, tag=f"t{h}", name=f"pq{h}")
                      for h in range(HPC)]
                pk = [ps.tile([128, QT], F32, tag=f"t{2 + h}", name=f"pk{h}")
                      for h in range(HPC)]
                pv = [ps.tile([128, HPC * D], F32, tag=f"t{4 + i}", name=f"pv{i}")
                      for i in range(4)]
                # v first: these matmuls run while the previous s-tile's rope
                # drains the q/k psum banks on DVE; x stays resident for q/k
                xts = []
                for mc in range(NMC):
                    xt = xp.tile([128, QT], BF16, tag="x", name="xt")
                    nc.sync.dma_start(out=xt, in_=xP[b, mc, st])
                    xts.append(xt)
                    for ss in range(4):
                        nc.tensor.matmul(
                            pv[ss],
                            xt[:, ss * 128 : (ss + 1) * 128], wv_t[mc],
                            start=(mc == 0), stop=(mc == NMC - 1),
                        )
                for ss in range(4):
                    nc.scalar.copy(out=v_ts[st][:, ss, :], in_=pv[ss])
                for mc in range(NMC):
                    for h in range(HPC):
                        nc.tensor.matmul(
                            pq[h], wq_t[mc][:, h * D : (h + 1) * D], xts[mc],
                            start=(mc == 0), stop=(mc == NMC - 1),
                        )
                        nc.tensor.matmul(
                            pk[h], wk_t[mc][:, h * D : (h + 1) * D], xts[mc],
                            start=(mc == 0), stop=(mc == NMC - 1),
                        )
                # rope: dst = src*cos + shifted(src)*sinN  (sign is in sinN).
                # PSUM reads must stay on DVE (Pool has no PSUM port); the
                # SBUF-only final add goes to the otherwise-idle Pool engine.
                cs = slice(st * QT, st * QT + QT)
                for h in range(HPC):
                    for src, dst in ((pq[h], qr_ts[st]), (pk[h], kr_ts[st])):
                        tmp = attn.tile([128, 2, QT], F32, tag="ropetmp")
                        nc.vector.tensor_mul(tmp[:, 0], src, cos_t[:, cs])
                        nc.vector.tensor_mul(tmp[0:64, 1], src[64:128], sin_t[0:64, cs])
                        nc.vector.tensor_mul(tmp[64:128, 1], src[0:64], sin_t[64:128, cs])
                        nc.vector.tensor_add(dst[:, h, :], tmp[:, 0], tmp[:, 1])

        # ---- attention for batch b; heads interleaved, PV trails by one
        # k-block; `filler` thunks keep the PE fed while ACT exps ----
        def attn_phase(b, par, qr_ts, kr_ts, v_ts, filler):
            for qt in range(NQT):
                n_kb = 4 * qt + 4
                po = [ps.tile([128, QT], F32, tag=f"t{4 + h}", name=f"po{h}")
                      for h in range(HPC)]
                acc = [accp.tile([128, QT], BF16, tag=f"acc{h}", name=f"acc{h}")
                       for h in range(HPC)]

                def flush(h, e, kb, o):
                    nc.tensor.matmul(
                        po[h][:, o:],
                        v_ts[kb // 4][:, kb % 4, h * D : (h + 1) * D], e[:, o:],
                        start=(kb == 0), stop=(kb == n_kb - 1),
                    )

                prev = {}
                for kb in range(n_kb):
                    diag = kb - (n_kb - 4)
                    o = max(diag, 0) * 128
                    cur = {}
                    for h in range(HPC):
                        pscore = ps.tile([128, QT], F32, tag=f"t{h}",
                                         name=f"pscore{h}")
                        nc.tensor.matmul(
                            pscore[:, o:],
                            kr_ts[kb // 4][:, h, (kb % 4) * 128 : (kb % 4 + 1) * 128],
                            qr_ts[qt][:, h, o:],
                            start=True, stop=True,
                        )
                        e = expp.tile([128, QT], BF16, tag=f"e{h}", name=f"e{h}",
                                      bufs=3)
                        nc.scalar.activation(
                            out=e[:, o:], in_=pscore[:, o:],
                            func=mybir.ActivationFunctionType.Exp, scale=SCALE,
                        )
                        if diag >= 0:
                            # only columns [o, o+128) of the block straddle
                            # the diagonal; q >= o+128 is fully allowed
                            nc.vector.tensor_mul(
                                e[:, o : o + 128], e[:, o : o + 128],
                                band_t[:, 384:512],
                            )
                        if kb == 0:
                            if qt == 0:
                                nc.vector.tensor_copy(out=acc[h], in_=e)
                        elif kb == 1 and qt > 0:
                            # fused init: both blocks are full-width off-diag
                            nc.vector.tensor_add(acc[h], prev[h][0], e)
                        else:
                            nc.vector.tensor_add(
                                acc[h][:, o:], acc[h][:, o:], e[:, o:]
                            )
                        cur[h] = (e, kb, o)
                    for h in range(HPC):
                        if prev:
                            flush(h, *prev[h])
                    emit_filler(filler, 3)
                    prev = cur
                for h in range(HPC):
                    flush(h, *prev[h])

                # row sums via ones-matmul with a FULL [128,128] ones lhsT:
                # same 512-cycle cost, but the sum arrives replicated across
                # all 128 partitions — the reciprocal is then already
                # broadcast and no pb outer-product matmul is needed.
                for h in range(HPC):
                    pS = ps.tile([128, QT], F32, tag=f"t{6 + h}", name=f"pS{h}")
                    nc.tensor.matmul(pS, ones_t, acc[h], start=True, stop=True)
                    recip = attn.tile([128, QT], BF16, tag="recip")
                    with nc.allow_low_precision(reason="bf16 recip, feeds bf16 mul"):
                        nc.vector.reciprocal(out=recip, in_=pS)
                    a32 = attn.tile([128, QT], BF16, tag="a32")
                    nc.scalar.copy(out=a32, in_=po[h])
                    aout = attn.tile([128, QT], BF16, tag="aout")
                    nc.gpsimd.tensor_mul(aout, a32, recip)
                    nc.sync.dma_start(
                        out=a2a_ins[par][b * NQT + qt, h * D : (h + 1) * D, :],
                        in_=aout,
                    )
                emit_filler(filler, 2)

        # ---- iteration chain: A2A(it) overlaps qkv(it+1); outproj(it-1)
        # is the attention filler of iteration it ----
        qr_ts = [qkv.tile([128, HPC, QT], BF16, tag=f"qr{st}", name=f"qr{st}")
                 for st in range(NQT)]
        kr_ts = [qkv.tile([128, HPC, QT], BF16, tag=f"kr{st}", name=f"kr{st}")
                 for st in range(NQT)]
        v_ts = [qkv.tile([128, 4, HPC * D], BF16, tag=f"v{st}", name=f"v{st}")
                for st in range(NQT)]

        for it in range(n_iters):
            par = it % 2
            filler = make_outproj_thunks() if it > 0 else []
            for b in range(B):
                qkv_phase(b, qr_ts, kr_ts, v_ts)
                attn_phase(b, par, qr_ts, kr_ts, v_ts, filler)
            emit_filler(filler, len(filler))
            _a2a(nc, a2a_ins[par], a2a_outs[par], single_core)
            for cc in range(NMC):
                nc.sync.dma_start(
                    out=recv_t[:, cc, :],
                    in_=a2a_outs[par][cc // 2, (cc % 2) * 128 : (cc % 2) * 128 + 128, :],
                )
        # epilogue: out-projection of the final iteration
        emit_filler(make_outproj_thunks(), NMC * NMC)


_CACHE = {}


def _get_built(n_iters=1):
    if n_iters not in _CACHE:
        _CACHE[n_iters] = build(n_iters)
    return _CACHE[n_iters]


def _fallback_numpy(x, w_qkv, w_out, mask):
    B_, S_, _ = x.shape
    qkv = x @ w_qkv
    qkv = qkv.reshape(B_, S_, 3, H, D).transpose(2, 0, 3, 1, 4)
    q, k, v = qkv[0], qkv[1], qkv[2]
    cosT, sinNT = _rope_tables()
    sinT = sinNT.copy()
    sinT[: D // 2] *= -1.0
    cos, sin = cosT.T[None, None], sinT.T[None, None]

    def rot(t):
        return np.concatenate([-t[..., D // 2 :], t[..., : D // 2]], axis=-1)

    q = q * cos + rot(q) * sin
    k = k * cos + rot(k) * sin
    score = np.einsum("bhqd,bhkd->bhqk", q, k) * SCALE
    score = np.where(mask == 0, -np.inf, score)
    score = score - score.max(axis=-1, keepdims=True)
    e = np.exp(score)
    attn = e / e.sum(axis=-1, keepdims=True)
    out = np.einsum("bhqk,bhkd->bhqd", attn, v)
    out = out.transpose(0, 2, 1, 3).reshape(B_, S_, H * D)
    return (out @ w_out).astype(np.float32)


def make_in_maps(x, w_qkv, w_out):
    cosT, sinNT = _rope_tables()
    band = _band_mask()
    # x pre-tiled: [B, mc, st, 128, 512], contiguous per tile
    xT = x.transpose(0, 2, 1)  # [B, DIM, S]
    xP = np.ascontiguousarray(
        xT.reshape(B, NMC, 128, NQT, QT).transpose(0, 1, 3, 2, 4)
    ).astype(NPBF16)
    # w_out pre-swizzled: [oc, p, cc, o] so each [128, 16*128] load is contiguous
    woP = np.ascontiguousarray(
        w_out.reshape(NMC, 128, NMC, 128).transpose(2, 1, 0, 3)
    ).astype(NPBF16)
    cosT = cosT.astype(NPBF16)
    sinNT = sinNT.astype(NPBF16)
    band = band.astype(NPBF16)
    in_maps = []
    for c in range(N_CORES):
        heads = [HPC * c + i for i in range(HPC)]

        def wslice(base):
            w = np.concatenate(
                [w_qkv[:, base + h * D : base + (h + 1) * D] for h in heads], axis=1
            )  # [DIM, 256]
            # -> [p, mc, 256] contiguous per partition
            return np.ascontiguousarray(
                w.reshape(NMC, 128, HPC * D).transpose(1, 0, 2)
            ).astype(NPBF16)

        in_maps.append(
            {
                "xP": xP,
                "wq": wslice(0),
                "wk": wslice(DIM),
                "wv": wslice(2 * DIM),
                "woP": woP,
                "cosT": cosT,
                "sinNT": sinNT,
                "band": band,
            }
        )
    return in_maps


def assemble_output(results):
    out = np.zeros((B, S, DIM), np.float32)
    for j in range(N_CORES):
        b, sq = j // NQT, j % NQT
        out[b, sq * QT : (sq + 1) * QT, :] = results[j]["outT"].T
    return out


def kernel(x, w_qkv, w_out, mask):
    x = np.asarray(x, dtype=np.float32)
    w_qkv = np.asarray(w_qkv, dtype=np.float32)
    w_out = np.asarray(w_out, dtype=np.float32)
    mask = np.asarray(mask)
    if not np.array_equal(mask != 0, np.tril(np.ones((S, S), bool))):
        return _fallback_numpy(x, w_qkv, w_out, mask)
    nc = _get_built(1)
    res = run_bass_kernel_spmd(nc, make_in_maps(x, w_qkv, w_out), list(range(N_CORES)))
    return assemble_output(res.results)
